# revision 8
# baseline (speedup 1.0000x reference)
"""Channel-attention kernel for Trainium2, SPMD across 8 NeuronCores.

Problem: x:[4,512,64,64] f32; q = wq@x+bq, k = wk@x+bk (Cq=64), v = wv@x+bv;
scores = q^T k -> [B,4096,4096]; attn = softmax(scores, -1);
out = v @ attn^T; y = gamma*out + x.

Sharding: 8 shards = 4 batches x 2 query-halves. Each core gets its batch's
x pre-rotated along the pixel axis so its 2048 queries sit in columns 0:2048
(softmax/AV are permutation-invariant over keys, so rotating keys/values is
harmless). This keeps the SPMD program identical on every core.

Per-core pipeline (v3 -- column-slab streamed, ACT-saturating):
  P1: x streams in as eight 512-pixel column slabs on two DMA queue sets
      (sync: channel blocks 0-1, gpsimd: blocks 2-3) writing straight into a
      persistent f32 x image.  Per slab: cast to fp8 (xp), QK-project in fp8
      DoubleRow (2 matmuls per slab instead of 8 bf16 ones), V-project the
      previous slab's two key-tile pairs, and issue group-0's score pairs +
      exp right behind.  The exp stream on ACT starts ~11us in and ideally
      never stops again.
  P2: three group slots.  Slot g runs scores(g+1)+exp(g+1) on PE/ACT while
      the PE drains AV(g) as four 16-matmul ct-chains (fp8e4 x fp8e5 DR,
      the peak-rate path), the denominator chain for group g+1 rides as a
      self-pacing burst at the slot's end, and the epilogue for (g, ct)
      (av*gamma/d on DVE, +gamma*bv +x on GpSimd from the f32 x still in
      SBUF) trails each ct-chain with y DMAs on the idle sync queue.
  Tail: AV(3) + epilogue only (scores/exp all done).

Residual precision: x is kept in f32 (no bf16 round-trip), so the visible
error of the gamma*attn + x path is tiny; the attention path runs in fp8
with a fixed exp bias of -4.
"""

import numpy as np

import concourse.bass as bass
import concourse.bacc as bacc
import concourse.mybir as mybir
import concourse.tile as tile
from concourse import bass_utils, masks

B, C, W, H = 4, 512, 64, 64
N = W * H          # 4096 pixels
CQ = 64            # query/key channels
NH = N // 2        # 2048 queries per core
NCORES = 8
F32 = mybir.dt.float32
BF16 = mybir.dt.bfloat16
FP8E4 = mybir.dt.float8e4
FP8E5 = mybir.dt.float8e5
DR = mybir.MatmulPerfMode.DoubleRow
VPAD = 528   # fp8 vT pair stride, %16 == 0
AF = mybir.ActivationFunctionType
MUL = mybir.AluOpType.mult
ADD = mybir.AluOpType.add

NJ = 16            # key-tile pairs
N_G = NH // 512    # 4 query groups per core
NS = 8             # x column slabs of 512 pixels


def _emit(tc, x, wq, wk, wv, bqk, bv, gamma, y):
    nc = tc.nc

    with (
        tc.tile_pool(name="const", bufs=1) as const,
        tc.tile_pool(name="data", bufs=1) as data,
        tc.tile_pool(name="wstg", bufs=1) as wstg,
    ):
        # ---- x slab DMAs first: sync queue carries channel blocks 0,1 ---
        xf = [data.tile([128, N], F32, tag=f"xf{r}", name=f"xf{r}")
              for r in range(4)]
        for s in range(NS):
            lo = s * 512
            nc.sync.dma_start(xf[0][:, lo:lo + 512], x[0:128, lo:lo + 512])
            nc.sync.dma_start(xf[1][:, lo:lo + 512],
                              x[128:256, lo:lo + 512])

        # ---- weight DMAs on the scalar queue ----------------------------
        bqk_s = const.tile([128, 1], F32, tag="bqk")
        bv_s = const.tile([1, C], F32, tag="bvs")
        g_s = const.tile([1, 1], F32, tag="gs")
        wqk_f = wstg.tile([128, C], F32, tag="wqkf")
        nc.scalar.dma_start(wqk_f[0:CQ, :], wq)
        nc.scalar.dma_start(wqk_f[CQ:128, :], wk)
        wvf = []
        for r in range(4):
            wf = wstg.tile([128, C], F32, tag=f"wvf{r}", name=f"wf{r}")
            nc.scalar.dma_start(wf[:], wv[r * 128:(r + 1) * 128, :])
            wvf.append(wf)
        nc.scalar.dma_start(bqk_s[:], bqk)
        nc.scalar.dma_start(bv_s[:], bv)
        nc.scalar.dma_start(g_s[:], gamma)

        # ---- constants (gpsimd memsets run before its x triggers) -------
        id_bf = const.tile([128, 128], BF16, tag="idb")
        masks.make_identity(nc, id_bf[:])
        ones_f32 = const.tile([1, 128], F32, tag="ones")
        nc.gpsimd.memset(ones_f32[:], 1.0)
        nbias = const.tile([128, 1], F32, tag="nbias")
        nc.gpsimd.memset(nbias[:], -4.0)
        onesP = const.tile([128, 32], FP8E4, tag="onesP")
        nc.gpsimd.memset(onesP[:], 1.0)

        # ---- persistent data --------------------------------------------
        xp = [data.tile([128, 2 * N], FP8E4, tag=f"xp{pc}", name=f"xp{pc}")
              for pc in range(2)]
        qkb = data.tile([128, N], BF16, tag="qkb")
        k2lo = data.tile([64, N], BF16, tag="k2lo")
        q2hi = data.tile([128, NH], BF16, tag="q2hi")
        vP = [data.tile([128, 2 * VPAD], FP8E4, tag=f"vP{j}", name=f"vP{j}")
              for j in range(NJ)]
        wqkT8 = [data.tile([128, 256], FP8E4, tag=f"wqkT8{pc}",
                           name=f"wqkT8{pc}")
                 for pc in range(2)]
        wvTp = [data.tile([128, 1024], FP8E4, tag=f"wvTp{pc}",
                          name=f"wvTp{pc}")
                for pc in range(2)]
        gones = const.tile([1, 128], BF16, tag="gones")
        gammab = const.tile([128, 1], F32, tag="gammab")
        gbv = const.tile([128, 4], F32, tag="gbv")

        def alloc_expP(g):
            return [data.tile([128, 1024], FP8E5, tag=f"expP{j}",
                              name=f"expP{j}_{g}", bufs=2)
                    for j in range(NJ)]

        with (
            tc.tile_pool(name="psSC", bufs=2, space="PSUM") as psSC,
            tc.tile_pool(name="psD", bufs=1, space="PSUM") as psD,
        ):

            def score_pair(expP_list, g, j):
                mA, mB = 2 * j, 2 * j + 1
                ps = psSC.tile([128, 1024], F32, tag="sc",
                               name=f"ps{g}_{j}")
                nc.tensor.matmul(
                    ps[:, 0:512], k2lo[:, mA * 128:(mA + 1) * 128],
                    qkb[0:CQ, g * 512:(g + 1) * 512],
                    start=True, stop=True,
                )
                nc.tensor.matmul(
                    ps[:, 512:1024],
                    qkb[CQ:128, mB * 128:(mB + 1) * 128],
                    q2hi[CQ:128, g * 512:(g + 1) * 512],
                    start=True, stop=True,
                )
                nc.scalar.activation(expP_list[j][:], ps[:], AF.Exp,
                                     bias=nbias[:])

            # ================= P1: slab-streamed prologue =================
            with (
                tc.tile_pool(name="psQK", bufs=1, space="PSUM") as psQK,
                tc.tile_pool(name="psV", bufs=2, space="PSUM") as psV,
            ):
                expP = alloc_expP(0)

                def v_pair(j, drain_eng):
                    # two key tiles; per-half PSUM so drains double-buffer
                    for half in range(2):
                        mt = 2 * j + half
                        ps = psV.tile([128, 512], F32, tag="v",
                                      name=f"vps{j}_{half}")
                        for pc in range(2):
                            lhx = xp[pc][:].rearrange(
                                "p (i n) -> p i n", i=2)[
                                :, :, mt * 128:(mt + 1) * 128]
                            wvr = wvTp[pc][:].rearrange(
                                "p (i n) -> p i n", i=2)
                            nc.tensor.matmul(
                                ps[:], lhx, wvr,
                                start=(pc == 0), stop=(pc == 1),
                                perf_mode=DR,
                            )
                        dst = vP[j][:, half * VPAD:half * VPAD + 512]
                        if drain_eng == "act":
                            nc.scalar.activation(dst, ps[:], AF.Copy)
                        else:
                            nc.vector.tensor_copy(dst, ps[:])

                def x_triggers(s):
                    lo = s * 512
                    nc.gpsimd.dma_start(xf[2][:, lo:lo + 512],
                                        x[256:384, lo:lo + 512])
                    nc.gpsimd.dma_start(xf[3][:, lo:lo + 512],
                                        x[384:512, lo:lo + 512])

                def slab_front(s):
                    """fp8 casts + fp8 DR QK + bias + partition-split"""
                    lo = s * 512
                    for r in range(4):
                        eng = nc.vector if r < 2 else nc.gpsimd
                        eng.tensor_copy(
                            xp[r // 2][:, (r % 2) * N + lo:
                                       (r % 2) * N + lo + 512],
                            xf[r][:, lo:lo + 512])
                    qps = psQK.tile([128, 512], F32, tag="qk",
                                    name=f"qps{s}")
                    for pc in range(2):
                        mv = xp[pc][:].rearrange(
                            "p (i n) -> p i n", i=2)[:, :, lo:lo + 512]
                        st = wqkT8[pc][:].rearrange(
                            "p (i n) -> p i n", i=2)
                        nc.tensor.matmul(qps[:], st, mv,
                                         start=(pc == 0), stop=(pc == 1),
                                         perf_mode=DR)
                    nc.vector.tensor_scalar_add(
                        qkb[:, lo:lo + 512], qps[:], bqk_s[:])
                    nc.gpsimd.dma_start(
                        k2lo[:, lo:lo + 512], qkb[CQ:128, lo:lo + 512])
                    if s < 4:
                        nc.gpsimd.dma_start(
                            q2hi[CQ:128, lo:lo + 512],
                            qkb[0:CQ, lo:lo + 512])

                # wq/wk prep first: ptq shares the psQK "qk" tag, so it must
                # be allocated before qps(0) (rotation order = dep order)
                x_triggers(0)
                x_triggers(1)
                wqkb = wstg.tile([128, C], BF16, tag="wqkb")
                nc.vector.tensor_copy(wqkb[:], wqk_f[:])
                ptq = [psQK.tile([128, 256], BF16, tag="qk",
                                 name=f"ptq{i}") for i in range(2)]
                for cc in range(4):
                    nc.tensor.transpose(
                        ptq[cc // 2][:, (cc % 2) * 128:(cc % 2) * 128 + 128],
                        wqkb[:, cc * 128:(cc + 1) * 128], id_bf[:])
                for pc in range(2):
                    nc.vector.tensor_copy(wqkT8[pc][:], ptq[pc][:])

                # -- slab 0 --
                slab_front(0)
                x_triggers(2)
                score_pair(expP, 0, 0)
                score_pair(expP, 0, 1)

                # -- slab 1 + wv prep (transposes on PE, fp8 copies on ACT,
                #    which is still ahead of the exp stream) --
                slab_front(1)
                x_triggers(3)
                wvb = [wstg.tile([128, C], BF16, tag=f"wvb{r}",
                                 name=f"wvb{r}") for r in range(4)]
                for r in range(4):
                    nc.vector.tensor_copy(wvb[r][:], wvf[r][:])
                for cc in range(4):
                    pt = psV.tile([128, C], BF16, tag="v", name=f"ptv{cc}")
                    for r in range(4):
                        nc.tensor.transpose(
                            pt[:, r * 128:(r + 1) * 128],
                            wvb[r][:, cc * 128:(cc + 1) * 128],
                            id_bf[:],
                        )
                    nc.scalar.activation(
                        wvTp[cc // 2][:, (cc % 2) * 512:(cc % 2) * 512 + 512],
                        pt[:], AF.Copy)
                # epilogue constants (needed only from slot 0 on)
                nc.vector.tensor_scalar_mul(gones[:], ones_f32[:], g_s[:])
                pg = psD.tile([128, 4], F32, tag="d", name="pg")
                nc.tensor.matmul(pg[:, 0:1], ones_f32[:], g_s[:],
                                 start=True, stop=True)
                nc.vector.tensor_copy(gammab[:], pg[:, 0:1])
                pbvT = psD.tile([128, 4], F32, tag="d", name="pbvT")
                for ct in range(4):
                    nc.tensor.matmul(
                        pbvT[:, ct:ct + 1],
                        bv_s[0:1, ct * 128:(ct + 1) * 128],
                        ones_f32[0:1, 0:1], start=True, stop=True)
                nc.vector.tensor_scalar_mul(gbv[:], pbvT[:], gammab[:])
                score_pair(expP, 0, 2)
                score_pair(expP, 0, 3)

                # -- slabs 2..7: steady state, v-pairs lag two slabs --
                for s in range(2, NS):
                    slab_front(s)
                    if s + 2 < NS:
                        x_triggers(s + 2)
                    for j in (2 * s - 4, 2 * s - 3):
                        v_pair(j, "dve")
                    score_pair(expP, 0, 2 * s)
                    score_pair(expP, 0, 2 * s + 1)
                # final v-pairs; route their PSUM drains through ACT so the
                # DVE queue isn't the long pole at the P1->slot0 boundary
                for j in (12, 13):
                    v_pair(j, "dve")
                for j in (14, 15):
                    v_pair(j, "act")

            # ============== P2: group slots + tail ========================
            ones_ap = onesP[:].rearrange("p (i n) -> p i n", i=2)[:, :, 0:1]

            def denom_burst(g, expP_list):
                dt = psD.tile([128, 512], F32, tag="d", name=f"d{g}")
                for j in range(NJ):
                    nc.tensor.matmul(
                        dt[0:1, :], ones_ap,
                        expP_list[j][:].rearrange("p (i n) -> p i n", i=2),
                        start=(j == 0), stop=(j == NJ - 1),
                        perf_mode=DR,
                    )
                return dt

            dt = denom_burst(0, expP)

            with (
                tc.tile_pool(name="psAV", bufs=3, space="PSUM") as psAV,
                tc.tile_pool(name="small", bufs=2) as small,
                tc.tile_pool(name="yout", bufs=2) as yout,
            ):
                for g in range(N_G):
                    nxt = alloc_expP(g + 1) if g + 1 < N_G else None
                    dr = small.tile([1, 512], BF16, tag="dr")
                    with nc.allow_low_precision(
                            reason="1/d in bf16; rescaled by gamma"):
                        nc.vector.reciprocal(dr[:], dt[0:1, :])
                    gdb = psAV.tile([128, 512], F32, tag="av",
                                    name=f"gdb{g}")
                    nc.tensor.matmul(gdb[:], gones[:], dr[:],
                                     start=True, stop=True)
                    gdbs = small.tile([128, 512], F32, tag="gdbs", bufs=2)
                    nc.vector.tensor_copy(gdbs[:], gdb[:])

                    gcols = slice(g * 512, (g + 1) * 512)
                    for ct in range(4):
                        if nxt is not None:
                            for j in range(ct * 4, ct * 4 + 4):
                                score_pair(nxt, g + 1, j)
                        av = psAV.tile([128, 512], F32, tag="av",
                                       name=f"av{g}_{ct}")
                        for j in range(NJ):
                            vst = vP[j][:].rearrange(
                                "p (i n) -> p i n", i=2)[
                                :, :, ct * 128:(ct + 1) * 128]
                            nc.tensor.matmul(
                                av[:], vst,
                                expP[j][:].rearrange("p (i n) -> p i n",
                                                     i=2),
                                start=(j == 0), stop=(j == NJ - 1),
                                perf_mode=DR,
                            )
                        tmp = yout.tile([128, 512], F32, tag="tmp")
                        nc.vector.tensor_tensor(tmp[:], av[:], gdbs[:], MUL)
                        yo = yout.tile([128, 512], F32, tag="yo")
                        # yo = (tmp + gamma*bv) + x   (x still f32 in SBUF)
                        nc.vector.scalar_tensor_tensor(
                            yo[:], tmp[:], gbv[:, ct:ct + 1],
                            xf[ct][:, gcols], ADD, ADD)
                        eng = nc.sync if ct % 2 == 0 else nc.gpsimd
                        eng.dma_start(
                            y[ct * 128:(ct + 1) * 128, gcols], yo[:])
                    if nxt is not None:
                        dt = denom_burst(g + 1, nxt)
                    expP = nxt


def build_nc():
    nc = bacc.Bacc("TRN2", target_bir_lowering=False, debug=False,
                   num_devices=NCORES)
    x = nc.dram_tensor("x", [C, N], F32, kind="ExternalInput")
    wq = nc.dram_tensor("wq", [CQ, C], F32, kind="ExternalInput")
    wk = nc.dram_tensor("wk", [CQ, C], F32, kind="ExternalInput")
    wv = nc.dram_tensor("wv", [C, C], F32, kind="ExternalInput")
    bqk = nc.dram_tensor("bqk", [128, 1], F32, kind="ExternalInput")
    bv = nc.dram_tensor("bv", [1, C], F32, kind="ExternalInput")
    gamma = nc.dram_tensor("gamma", [1, 1], F32, kind="ExternalInput")
    y = nc.dram_tensor("y", [C, NH], F32, kind="ExternalOutput")
    with tile.TileContext(nc) as tc:
        _emit(tc, x.ap(), wq.ap(), wk.ap(), wv.ap(), bqk.ap(), bv.ap(),
              gamma.ap(), y.ap())
    nc.compile()
    return nc


def make_in_maps(inputs):
    xf = np.ascontiguousarray(
        np.asarray(inputs["x"], dtype=np.float32).reshape(B, C, N))
    wq = np.ascontiguousarray(np.asarray(inputs["wq"], dtype=np.float32))
    wk = np.ascontiguousarray(np.asarray(inputs["wk"], dtype=np.float32))
    wv = np.ascontiguousarray(np.asarray(inputs["wv"], dtype=np.float32))
    bqk = np.concatenate([
        np.asarray(inputs["bq"], dtype=np.float32),
        np.asarray(inputs["bk"], dtype=np.float32),
    ]).reshape(128, 1)
    bv = np.asarray(inputs["bv"], dtype=np.float32).reshape(1, C)
    gamma = np.asarray(inputs["gamma"], dtype=np.float32).reshape(1, 1)
    in_maps = []
    for i in range(NCORES):
        b, h = divmod(i, 2)
        xr = np.roll(xf[b], -h * NH, axis=1) if h else xf[b]
        in_maps.append({
            "x": np.ascontiguousarray(xr), "wq": wq, "wk": wk, "wv": wv,
            "bqk": bqk, "bv": bv, "gamma": gamma,
        })
    return in_maps


_NC = None


def _get_nc():
    global _NC
    if _NC is None:
        _NC = build_nc()
    return _NC


def kernel(**inputs):
    nc = _get_nc()
    in_maps = make_in_maps(inputs)
    res = bass_utils.run_bass_kernel_spmd(nc, in_maps, core_ids=list(range(NCORES)))
    yf = np.empty((B, C, N), dtype=np.float32)
    for i in range(NCORES):
        b, h = divmod(i, 2)
        yf[b][:, h * NH:(h + 1) * NH] = res.results[i]["y"]
    return yf.reshape(B, C, W, H)


# revision 15
# speedup vs baseline: 1.0438x; 1.0438x over previous
"""Channel-attention kernel for Trainium2, SPMD across 8 NeuronCores.

Problem: x:[4,512,64,64] f32; q = wq@x+bq, k = wk@x+bk (Cq=64), v = wv@x+bv;
scores = q^T k -> [B,4096,4096]; attn = softmax(scores, -1);
out = v @ attn^T; y = gamma*out + x.

Sharding: 8 shards = 4 batches x 2 query-halves. Each core gets its batch's
x pre-rotated along the pixel axis so its 2048 queries sit in columns 0:2048
(softmax/AV are permutation-invariant over keys, so rotating keys/values is
harmless). This keeps the SPMD program identical on every core.

Per-core pipeline (v4 -- column-slab streamed, ACT-saturating):
  P1: x streams in as eight 512-pixel column slabs; slab 0's four channel
      blocks go out on four different queues (sync/gpsimd/scalar/vector) so
      the first QK chunk lands ~4us after the preamble, later slabs ride
      sync (blocks 0,1) + gpsimd (blocks 2,3).  Per slab: fp8 cast on DVE,
      QK-projection in fp8 DoubleRow, bias-add on ACT (rides between exps),
      V-projection (lagging two slabs, per-half PSUM drains on DVE), and
      group-0 score pairs + exp + a j-by-j denominator chain right behind.
      Weight transposes run on the PE straight from f32 (no bf16 casts).
  P2: three group slots.  Slot g streams scores(g+1)+exp(g+1) finely
      interleaved with the four AV(g) ct-chains in 4-matmul chunks (so the
      in-order PE queue never parks on a not-yet-ready score PSUM buffer),
      the denominator chain for g+1 rides j-by-j two pairs behind the exp
      stream, and the (g, ct) epilogue (av*gamma/d on DVE, +gamma*bv +x on
      DVE from the f32 x still in SBUF) trails each ct-chain with y DMAs on
      the idle sync/gpsimd queues.
  Tail: AV(3) + epilogue only.

Residual precision: x is kept in f32 (no bf16 round-trip), so the visible
error of the gamma*attn + x path is tiny; the attention path runs in fp8
with a fixed exp bias of -4.
"""

import numpy as np

import concourse.bass as bass
import concourse.bacc as bacc
import concourse.mybir as mybir
import concourse.tile as tile
from concourse import bass_utils, masks

B, C, W, H = 4, 512, 64, 64
N = W * H          # 4096 pixels
CQ = 64            # query/key channels
NH = N // 2        # 2048 queries per core
NCORES = 8
F32 = mybir.dt.float32
BF16 = mybir.dt.bfloat16
FP8E4 = mybir.dt.float8e4
FP8E5 = mybir.dt.float8e5
DR = mybir.MatmulPerfMode.DoubleRow
VPAD = 528   # fp8 vT pair stride, %16 == 0
AF = mybir.ActivationFunctionType
MUL = mybir.AluOpType.mult
ADD = mybir.AluOpType.add

NJ = 16            # key-tile pairs
N_G = NH // 512    # 4 query groups per core
NS = 8             # x column slabs of 512 pixels


def _emit(tc, x, wq, wk, wv, bqk, bv, gamma, y):
    nc = tc.nc

    with (
        tc.tile_pool(name="const", bufs=1) as const,
        tc.tile_pool(name="data", bufs=1) as data,
        tc.tile_pool(name="wstg", bufs=1) as wstg,
    ):
        xf = [data.tile([128, N], F32, tag=f"xf{r}", name=f"xf{r}")
              for r in range(4)]

        # ---- slab 0 on four queues, then the rest ------------------------
        nc.sync.dma_start(xf[0][:, 0:512], x[0:128, 0:512])
        nc.gpsimd.dma_start(xf[1][:, 0:512], x[128:256, 0:512])
        nc.scalar.dma_start(xf[2][:, 0:512], x[256:384, 0:512])
        nc.scalar.dma_start(xf[3][:, 0:512], x[384:512, 0:512])
        for s in range(1, NS):
            lo = s * 512
            nc.sync.dma_start(xf[0][:, lo:lo + 512], x[0:128, lo:lo + 512])
            nc.sync.dma_start(xf[1][:, lo:lo + 512],
                              x[128:256, lo:lo + 512])

        # ---- weight DMAs on the scalar queue -----------------------------
        bqk_s = const.tile([128, 1], F32, tag="bqk")
        bv_s = const.tile([1, C], F32, tag="bvs")
        g_s = const.tile([1, 1], F32, tag="gs")
        wqk_f = wstg.tile([128, C], F32, tag="wqkf")
        nc.scalar.dma_start(wqk_f[0:CQ, :], wq)
        nc.scalar.dma_start(wqk_f[CQ:128, :], wk)
        nc.scalar.dma_start(bqk_s[:], bqk)
        nc.scalar.dma_start(bv_s[:], bv)
        nc.scalar.dma_start(g_s[:], gamma)
        wvf = []
        for r in range(4):
            wf = wstg.tile([128, C], F32, tag=f"wvf{r}", name=f"wf{r}")
            nc.scalar.dma_start(wf[:], wv[r * 128:(r + 1) * 128, :])
            wvf.append(wf)

        # ---- constants (gpsimd memsets, before its x triggers) -----------
        id_bf = const.tile([128, 128], BF16, tag="idb")
        masks.make_identity(nc, id_bf[:])
        id_f32 = const.tile([128, 128], F32, tag="idf")
        masks.make_identity(nc, id_f32[:])
        ones_f32 = const.tile([1, 128], F32, tag="ones")
        nc.gpsimd.memset(ones_f32[:], 1.0)
        nbias = const.tile([128, 1], F32, tag="nbias")
        nc.gpsimd.memset(nbias[:], -4.0)
        onesP = const.tile([128, 32], FP8E4, tag="onesP")
        nc.gpsimd.memset(onesP[:], 1.0)

        # remaining x triggers for channel blocks 2,3 on gpsimd
        for s in range(1, NS):
            lo = s * 512
            nc.gpsimd.dma_start(xf[2][:, lo:lo + 512],
                                x[256:384, lo:lo + 512])
            nc.gpsimd.dma_start(xf[3][:, lo:lo + 512],
                                x[384:512, lo:lo + 512])

        # ---- persistent data ---------------------------------------------
        xp = [data.tile([128, 2 * N], FP8E4, tag=f"xp{pc}", name=f"xp{pc}")
              for pc in range(2)]
        qkb = data.tile([128, N], BF16, tag="qkb")
        k2lo = data.tile([64, N], BF16, tag="k2lo")
        q2hi = data.tile([128, NH], BF16, tag="q2hi")
        vP = [data.tile([128, 2 * VPAD], FP8E4, tag=f"vP{j}", name=f"vP{j}")
              for j in range(NJ)]
        wqkT8 = [data.tile([128, 256], FP8E4, tag=f"wqkT8{pc}",
                           name=f"wqkT8{pc}")
                 for pc in range(2)]
        wvTp = [data.tile([128, 1024], FP8E4, tag=f"wvTp{pc}",
                          name=f"wvTp{pc}")
                for pc in range(2)]
        gones = const.tile([1, 128], BF16, tag="gones")
        gammab = const.tile([128, 1], F32, tag="gammab")
        gbv = const.tile([128, 4], F32, tag="gbv")

        def alloc_expP(g):
            return [data.tile([128, 1024], FP8E5, tag=f"expP{j}",
                              name=f"expP{j}_{g}", bufs=2)
                    for j in range(NJ)]

        with (
            tc.tile_pool(name="psSC", bufs=2, space="PSUM") as psSC,
            tc.tile_pool(name="psD", bufs=1, space="PSUM") as psD,
        ):
            ones_ap = onesP[:].rearrange("p (i n) -> p i n", i=2)[:, :, 0:1]

            def score_pair(expP_list, g, j):
                mA, mB = 2 * j, 2 * j + 1
                ps = psSC.tile([128, 1024], F32, tag="sc",
                               name=f"ps{g}_{j}")
                nc.tensor.matmul(
                    ps[:, 0:512], k2lo[:, mA * 128:(mA + 1) * 128],
                    qkb[0:CQ, g * 512:(g + 1) * 512],
                    start=True, stop=True,
                )
                nc.tensor.matmul(
                    ps[:, 512:1024],
                    qkb[CQ:128, mB * 128:(mB + 1) * 128],
                    q2hi[CQ:128, g * 512:(g + 1) * 512],
                    start=True, stop=True,
                )
                nc.scalar.activation(expP_list[j][:], ps[:], AF.Exp,
                                     bias=nbias[:])

            def dn_link(dt, expP_list, j):
                nc.tensor.matmul(
                    dt[0:1, :], ones_ap,
                    expP_list[j][:].rearrange("p (i n) -> p i n", i=2),
                    start=(j == 0), stop=(j == NJ - 1), perf_mode=DR,
                )

            # ================= P1: slab-streamed prologue =================
            with (
                tc.tile_pool(name="psQK", bufs=1, space="PSUM") as psQK,
                tc.tile_pool(name="psV", bufs=2, space="PSUM") as psV,
                tc.tile_pool(name="vstg", bufs=4) as vstg,
            ):
                expP = alloc_expP(0)

                def v_pair(j):
                    # two key tiles.  PSUM is drained by on-chip DMA (f32,
                    # rides the idle sync/gpsimd queues) and the fp8 cast
                    # runs SBUF->SBUF on DVE in its fast 2x mode.
                    for half in range(2):
                        mt = 2 * j + half
                        ps = psV.tile([128, 512], F32, tag="v",
                                      name=f"vps{j}_{half}")
                        for pc in range(2):
                            lhx = xp[pc][:].rearrange(
                                "p (i n) -> p i n", i=2)[
                                :, :, mt * 128:(mt + 1) * 128]
                            wvr = wvTp[pc][:].rearrange(
                                "p (i n) -> p i n", i=2)
                            nc.tensor.matmul(
                                ps[:], lhx, wvr,
                                start=(pc == 0), stop=(pc == 1),
                                perf_mode=DR,
                            )
                        nc.vector.tensor_copy(
                            vP[j][:, half * VPAD:half * VPAD + 512], ps[:])

                def slab_front(s):
                    """fp8 casts (DVE) + fp8 DR QK + bias on ACT + splits"""
                    lo = s * 512
                    for r in range(4):
                        nc.vector.tensor_copy(
                            xp[r // 2][:, (r % 2) * N + lo:
                                       (r % 2) * N + lo + 512],
                            xf[r][:, lo:lo + 512])
                    qps = psQK.tile([128, 512], F32, tag="qk",
                                    name=f"qps{s}")
                    for pc in range(2):
                        mv = xp[pc][:].rearrange(
                            "p (i n) -> p i n", i=2)[:, :, lo:lo + 512]
                        st = wqkT8[pc][:].rearrange(
                            "p (i n) -> p i n", i=2)
                        nc.tensor.matmul(qps[:], st, mv,
                                         start=(pc == 0), stop=(pc == 1),
                                         perf_mode=DR)
                    # bias-add + bf16 cast on ACT (rides between exps)
                    nc.scalar.activation(qkb[:, lo:lo + 512], qps[:],
                                         AF.Identity, bias=bqk_s[:])
                    nc.gpsimd.dma_start(
                        k2lo[:, lo:lo + 512], qkb[CQ:128, lo:lo + 512])
                    if s < 4:
                        nc.gpsimd.dma_start(
                            q2hi[CQ:128, lo:lo + 512],
                            qkb[0:CQ, lo:lo + 512])

                # wq/wk transposed straight from f32; ptq shares the psQK
                # "qk" tag so it must be allocated before qps(0)
                ptq = [psQK.tile([128, 256], F32, tag="qk",
                                 name=f"ptq{i}") for i in range(2)]
                for cc in range(4):
                    nc.tensor.transpose(
                        ptq[cc // 2][:, (cc % 2) * 128:(cc % 2) * 128 + 128],
                        wqk_f[:, cc * 128:(cc + 1) * 128], id_f32[:])
                for pc in range(2):
                    nc.vector.tensor_copy(wqkT8[pc][:], ptq[pc][:])

                # -- slab 0 --
                slab_front(0)
                score_pair(expP, 0, 0)
                score_pair(expP, 0, 1)

                # -- slab 1 + wv prep (PE transposes from f32; fp8 copies
                #    on DVE) + epilogue constants --
                slab_front(1)
                for cc in range(4):
                    pt = psV.tile([128, C], F32, tag="v", name=f"ptv{cc}")
                    for r in range(4):
                        nc.tensor.transpose(
                            pt[:, r * 128:(r + 1) * 128],
                            wvf[r][:, cc * 128:(cc + 1) * 128],
                            id_f32[:],
                        )
                    nc.vector.tensor_copy(
                        wvTp[cc // 2][:, (cc % 2) * 512:(cc % 2) * 512 + 512],
                        pt[:])
                nc.vector.tensor_scalar_mul(gones[:], ones_f32[:], g_s[:])
                pg = psD.tile([128, 4], F32, tag="d", name="pg")
                nc.tensor.matmul(pg[:, 0:1], ones_f32[:], g_s[:],
                                 start=True, stop=True)
                nc.vector.tensor_copy(gammab[:], pg[:, 0:1])
                pbvT = psD.tile([128, 4], F32, tag="d", name="pbvT")
                for ct in range(4):
                    nc.tensor.matmul(
                        pbvT[:, ct:ct + 1],
                        bv_s[0:1, ct * 128:(ct + 1) * 128],
                        ones_f32[0:1, 0:1], start=True, stop=True)
                nc.vector.tensor_scalar_mul(gbv[:], pbvT[:], gammab[:])
                score_pair(expP, 0, 2)
                score_pair(expP, 0, 3)

                # -- slabs 2..7: steady state; v-pairs and the g0 denom
                #    chain lag two slabs/pairs behind --
                dt = psD.tile([128, 512], F32, tag="d", name="d0")
                for s in range(2, NS):
                    slab_front(s)
                    for j in (2 * s - 4, 2 * s - 3):
                        v_pair(j)
                    score_pair(expP, 0, 2 * s)
                    score_pair(expP, 0, 2 * s + 1)
                    dn_link(dt, expP, 2 * s - 4)
                    dn_link(dt, expP, 2 * s - 3)
                for j in (12, 13, 14, 15):
                    v_pair(j)
                    dn_link(dt, expP, j)

            # ============== P2: group slots + tail ========================
            with (
                tc.tile_pool(name="psAV", bufs=3, space="PSUM") as psAV,
                tc.tile_pool(name="small", bufs=2) as small,
                tc.tile_pool(name="yout", bufs=2) as yout,
            ):
                for g in range(N_G):
                    nxt = alloc_expP(g + 1) if g + 1 < N_G else None
                    dr = small.tile([1, 512], BF16, tag="dr")
                    with nc.allow_low_precision(
                            reason="1/d in bf16; rescaled by gamma"):
                        nc.vector.reciprocal(dr[:], dt[0:1, :])
                    gdb = psAV.tile([128, 512], F32, tag="av",
                                    name=f"gdb{g}")
                    nc.tensor.matmul(gdb[:], gones[:], dr[:],
                                     start=True, stop=True)
                    gdbs = small.tile([128, 512], F32, tag="gdbs", bufs=2)
                    nc.vector.tensor_copy(gdbs[:], gdb[:])
                    if nxt is not None:
                        dt = psD.tile([128, 512], F32, tag="d",
                                      name=f"d{g + 1}")

                    gcols = slice(g * 512, (g + 1) * 512)
                    av = None
                    for jj in range(NJ):
                        # next group's score/exp stream, finely interleaved
                        if nxt is not None:
                            score_pair(nxt, g + 1, jj)
                            if jj >= 2:
                                dn_link(dt, nxt, jj - 2)
                        # AV(g): four-matmul chunk of the current ct-chain
                        ct = jj // 4
                        if jj % 4 == 0:
                            av = psAV.tile([128, 512], F32, tag="av",
                                           name=f"av{g}_{ct}")
                        for j in range((jj % 4) * 4, (jj % 4) * 4 + 4):
                            vst = vP[j][:].rearrange(
                                "p (i n) -> p i n", i=2)[
                                :, :, ct * 128:(ct + 1) * 128]
                            nc.tensor.matmul(
                                av[:], vst,
                                expP[j][:].rearrange("p (i n) -> p i n",
                                                     i=2),
                                start=(j == 0), stop=(j == NJ - 1),
                                perf_mode=DR,
                            )
                        if jj % 4 == 3:
                            tmp = yout.tile([128, 512], F32, tag="tmp")
                            nc.vector.tensor_tensor(tmp[:], av[:],
                                                    gdbs[:], MUL)
                            yo = yout.tile([128, 512], F32, tag="yo")
                            # yo = (tmp + gamma*bv) + x   (x f32 in SBUF)
                            nc.vector.scalar_tensor_tensor(
                                yo[:], tmp[:], gbv[:, ct:ct + 1],
                                xf[ct][:, gcols], ADD, ADD)
                            eng = nc.sync if ct % 2 == 0 else nc.gpsimd
                            eng.dma_start(
                                y[ct * 128:(ct + 1) * 128, gcols], yo[:])
                    if nxt is not None:
                        dn_link(dt, nxt, 14)
                        dn_link(dt, nxt, 15)
                    expP = nxt


def build_nc():
    nc = bacc.Bacc("TRN2", target_bir_lowering=False, debug=False,
                   num_devices=NCORES)
    x = nc.dram_tensor("x", [C, N], F32, kind="ExternalInput")
    wq = nc.dram_tensor("wq", [CQ, C], F32, kind="ExternalInput")
    wk = nc.dram_tensor("wk", [CQ, C], F32, kind="ExternalInput")
    wv = nc.dram_tensor("wv", [C, C], F32, kind="ExternalInput")
    bqk = nc.dram_tensor("bqk", [128, 1], F32, kind="ExternalInput")
    bv = nc.dram_tensor("bv", [1, C], F32, kind="ExternalInput")
    gamma = nc.dram_tensor("gamma", [1, 1], F32, kind="ExternalInput")
    y = nc.dram_tensor("y", [C, NH], F32, kind="ExternalOutput")
    with tile.TileContext(nc) as tc:
        _emit(tc, x.ap(), wq.ap(), wk.ap(), wv.ap(), bqk.ap(), bv.ap(),
              gamma.ap(), y.ap())
    nc.compile()
    return nc


def make_in_maps(inputs):
    xf = np.ascontiguousarray(
        np.asarray(inputs["x"], dtype=np.float32).reshape(B, C, N))
    wq = np.ascontiguousarray(np.asarray(inputs["wq"], dtype=np.float32))
    wk = np.ascontiguousarray(np.asarray(inputs["wk"], dtype=np.float32))
    wv = np.ascontiguousarray(np.asarray(inputs["wv"], dtype=np.float32))
    bqk = np.concatenate([
        np.asarray(inputs["bq"], dtype=np.float32),
        np.asarray(inputs["bk"], dtype=np.float32),
    ]).reshape(128, 1)
    bv = np.asarray(inputs["bv"], dtype=np.float32).reshape(1, C)
    gamma = np.asarray(inputs["gamma"], dtype=np.float32).reshape(1, 1)
    in_maps = []
    for i in range(NCORES):
        b, h = divmod(i, 2)
        xr = np.roll(xf[b], -h * NH, axis=1) if h else xf[b]
        in_maps.append({
            "x": np.ascontiguousarray(xr), "wq": wq, "wk": wk, "wv": wv,
            "bqk": bqk, "bv": bv, "gamma": gamma,
        })
    return in_maps


_NC = None


def _get_nc():
    global _NC
    if _NC is None:
        _NC = build_nc()
    return _NC


def kernel(**inputs):
    nc = _get_nc()
    in_maps = make_in_maps(inputs)
    res = bass_utils.run_bass_kernel_spmd(nc, in_maps, core_ids=list(range(NCORES)))
    yf = np.empty((B, C, N), dtype=np.float32)
    for i in range(NCORES):
        b, h = divmod(i, 2)
        yf[b][:, h * NH:(h + 1) * NH] = res.results[i]["y"]
    return yf.reshape(B, C, W, H)


# revision 19
# speedup vs baseline: 1.1960x; 1.1458x over previous
"""Channel-attention kernel for Trainium2, SPMD across 8 NeuronCores.

Problem: x:[4,512,64,64] f32; q = wq@x+bq, k = wk@x+bk (Cq=64), v = wv@x+bv;
scores = q^T k -> [B,4096,4096]; attn = softmax(scores, -1);
out = v @ attn^T; y = gamma*out + x.

Sharding: 8 shards = 4 batches x 2 query-halves. Each core gets its batch's
x pre-rotated along the pixel axis so its 2048 queries sit in columns 0:2048
(softmax/AV are permutation-invariant over keys, so rotating keys/values is
harmless). This keeps the SPMD program identical on every core.

Per-core pipeline (v4 -- column-slab streamed, ACT-saturating):
  P1: x streams in as eight 512-pixel column slabs; slab 0's four channel
      blocks go out on four different queues (sync/gpsimd/scalar/vector) so
      the first QK chunk lands ~4us after the preamble, later slabs ride
      sync (blocks 0,1) + gpsimd (blocks 2,3).  Per slab: fp8 cast on DVE,
      QK-projection in fp8 DoubleRow, bias-add on ACT (rides between exps),
      V-projection (lagging two slabs, per-half PSUM drains on DVE), and
      group-0 score pairs + exp + a j-by-j denominator chain right behind.
      Weight transposes run on the PE straight from f32 (no bf16 casts).
  P2: three group slots.  Slot g streams scores(g+1)+exp(g+1) finely
      interleaved with the four AV(g) ct-chains in 4-matmul chunks (so the
      in-order PE queue never parks on a not-yet-ready score PSUM buffer),
      the denominator chain for g+1 rides j-by-j two pairs behind the exp
      stream, and the (g, ct) epilogue (av*gamma/d on DVE, +gamma*bv +x on
      DVE from the f32 x still in SBUF) trails each ct-chain with y DMAs on
      the idle sync/gpsimd queues.
  Tail: AV(3) + epilogue only.

Residual precision: x is kept in f32 (no bf16 round-trip), so the visible
error of the gamma*attn + x path is tiny; the attention path runs in fp8
with a fixed exp bias of -4.
"""

import numpy as np

import concourse.bass as bass
import concourse.bacc as bacc
import concourse.mybir as mybir
import concourse.tile as tile
from concourse import bass_utils, masks

B, C, W, H = 4, 512, 64, 64
N = W * H          # 4096 pixels
CQ = 64            # query/key channels
NH = N // 2        # 2048 queries per core
NCORES = 8
F32 = mybir.dt.float32
BF16 = mybir.dt.bfloat16
FP8E4 = mybir.dt.float8e4
FP8E5 = mybir.dt.float8e5
DR = mybir.MatmulPerfMode.DoubleRow
VPAD = 528   # fp8 vT pair stride, %16 == 0
AF = mybir.ActivationFunctionType
MUL = mybir.AluOpType.mult
ADD = mybir.AluOpType.add

NJ = 16            # key-tile pairs
N_G = NH // 512    # 4 query groups per core
NS = 8             # x column slabs of 512 pixels


def _emit(tc, x, wq, wk, wv, bqk, bv, gamma, y):
    nc = tc.nc

    with (
        tc.tile_pool(name="const", bufs=1) as const,
        tc.tile_pool(name="data", bufs=1) as data,
        tc.tile_pool(name="wstg", bufs=1) as wstg,
    ):
        xf = [data.tile([128, N], F32, tag=f"xf{r}", name=f"xf{r}")
              for r in range(4)]

        # ---- slab 0 spread over three DMA queues; wq/wk first ------------
        nc.sync.dma_start(xf[0][:, 0:512], x[0:128, 0:512])
        nc.gpsimd.dma_start(xf[1][:, 0:512], x[128:256, 0:512])
        nc.gpsimd.dma_start(xf[3][:, 0:512], x[384:512, 0:512])
        bqk_s = const.tile([128, 1], F32, tag="bqk")
        bv_s = const.tile([1, C], F32, tag="bvs")
        g_s = const.tile([1, 1], F32, tag="gs")
        wqk_f = wstg.tile([128, C], F32, tag="wqkf")
        nc.scalar.dma_start(wqk_f[0:CQ, :], wq)
        nc.scalar.dma_start(wqk_f[CQ:128, :], wk)
        nc.scalar.dma_start(xf[2][:, 0:512], x[256:384, 0:512])
        nc.scalar.dma_start(bqk_s[:], bqk)
        nc.scalar.dma_start(bv_s[:], bv)
        nc.scalar.dma_start(g_s[:], gamma)
        wvf = []
        for r in range(4):
            wf = wstg.tile([128, C], F32, tag=f"wvf{r}", name=f"wf{r}")
            nc.scalar.dma_start(wf[:], wv[r * 128:(r + 1) * 128, :])
            wvf.append(wf)
        # sync prefetches slabs 1,2 (blocks 0,1); later slabs are issued
        # inside slab_front with 2-slab lookahead so the k2lo/q2hi copies
        # interleave without head-of-line blocking the x stream
        for s in (1, 2):
            lo = s * 512
            nc.sync.dma_start(xf[0][:, lo:lo + 512], x[0:128, lo:lo + 512])
            nc.sync.dma_start(xf[1][:, lo:lo + 512],
                              x[128:256, lo:lo + 512])

        # ---- constants (gpsimd memsets, before its x triggers) -----------
        id_bf = const.tile([128, 128], BF16, tag="idb")
        masks.make_identity(nc, id_bf[:])
        id_f32 = const.tile([128, 128], F32, tag="idf")
        masks.make_identity(nc, id_f32[:])
        ones_f32 = const.tile([1, 128], F32, tag="ones")
        nc.gpsimd.memset(ones_f32[:], 1.0)
        nbias = const.tile([128, 1], F32, tag="nbias")
        nc.gpsimd.memset(nbias[:], -4.0)
        onesP = const.tile([128, 32], FP8E4, tag="onesP")
        nc.gpsimd.memset(onesP[:], 1.0)

        # remaining x triggers for channel blocks 2,3 ride gpsimd, which
        # carries no other work until the epilogue y DMAs
        for s in range(1, NS):
            lo = s * 512
            nc.gpsimd.dma_start(xf[2][:, lo:lo + 512],
                                x[256:384, lo:lo + 512])
            nc.gpsimd.dma_start(xf[3][:, lo:lo + 512],
                                x[384:512, lo:lo + 512])

        # ---- persistent data ---------------------------------------------
        xp = [data.tile([128, 2 * N], FP8E4, tag=f"xp{pc}", name=f"xp{pc}")
              for pc in range(2)]
        qkb = data.tile([128, N], BF16, tag="qkb")
        k2lo = data.tile([64, N], BF16, tag="k2lo")
        q2hi = data.tile([128, NH], BF16, tag="q2hi")
        vP = [data.tile([128, 2 * VPAD], FP8E4, tag=f"vP{j}", name=f"vP{j}")
              for j in range(NJ)]
        wqkT8 = [data.tile([128, 256], FP8E4, tag=f"wqkT8{pc}",
                           name=f"wqkT8{pc}")
                 for pc in range(2)]
        wvTp = [data.tile([128, 1024], FP8E4, tag=f"wvTp{pc}",
                          name=f"wvTp{pc}")
                for pc in range(2)]
        gones = const.tile([1, 128], F32, tag="gones")
        gammab = const.tile([128, 1], F32, tag="gammab")
        gbv = const.tile([128, 4], F32, tag="gbv")

        def alloc_expP(g):
            return [data.tile([128, 1024], FP8E5, tag=f"expP{j}",
                              name=f"expP{j}_{g}", bufs=2)
                    for j in range(NJ)]

        with (
            tc.tile_pool(name="psSC", bufs=2, space="PSUM") as psSC,
            tc.tile_pool(name="psD", bufs=1, space="PSUM") as psD,
        ):
            ones_ap = onesP[:].rearrange("p (i n) -> p i n", i=2)[:, :, 0:1]

            def score_pair(expP_list, g, j):
                mA, mB = 2 * j, 2 * j + 1
                ps = psSC.tile([128, 1024], F32, tag="sc",
                               name=f"ps{g}_{j}")
                nc.tensor.matmul(
                    ps[:, 0:512], k2lo[:, mA * 128:(mA + 1) * 128],
                    qkb[0:CQ, g * 512:(g + 1) * 512],
                    start=True, stop=True,
                )
                nc.tensor.matmul(
                    ps[:, 512:1024],
                    qkb[CQ:128, mB * 128:(mB + 1) * 128],
                    q2hi[CQ:128, g * 512:(g + 1) * 512],
                    start=True, stop=True,
                )
                nc.scalar.activation(expP_list[j][:], ps[:], AF.Exp,
                                     bias=nbias[:])

            def dn_link(dt, expP_list, j):
                nc.tensor.matmul(
                    dt[0:1, :], ones_ap,
                    expP_list[j][:].rearrange("p (i n) -> p i n", i=2),
                    start=(j == 0), stop=(j == NJ - 1), perf_mode=DR,
                )

            # ================= P1: slab-streamed prologue =================
            with (
                tc.tile_pool(name="psQK", bufs=1, space="PSUM") as psQK,
                tc.tile_pool(name="psV", bufs=2, space="PSUM") as psV,
                tc.tile_pool(name="vstg", bufs=4) as vstg,
            ):
                expP = alloc_expP(0)

                def v_pair(j):
                    # two key tiles.  PSUM is drained by on-chip DMA (f32,
                    # rides the idle sync/gpsimd queues) and the fp8 cast
                    # runs SBUF->SBUF on DVE in its fast 2x mode.
                    for half in range(2):
                        mt = 2 * j + half
                        ps = psV.tile([128, 512], F32, tag="v",
                                      name=f"vps{j}_{half}")
                        for pc in range(2):
                            lhx = xp[pc][:].rearrange(
                                "p (i n) -> p i n", i=2)[
                                :, :, mt * 128:(mt + 1) * 128]
                            wvr = wvTp[pc][:].rearrange(
                                "p (i n) -> p i n", i=2)
                            nc.tensor.matmul(
                                ps[:], lhx, wvr,
                                start=(pc == 0), stop=(pc == 1),
                                perf_mode=DR,
                            )
                        nc.vector.tensor_copy(
                            vP[j][:, half * VPAD:half * VPAD + 512], ps[:])

                def slab_front(s):
                    """fp8 casts (DVE) + fp8 DR QK + bias on ACT + splits"""
                    lo = s * 512
                    if 1 <= s and s + 2 < NS:
                        nlo = (s + 2) * 512
                        nc.sync.dma_start(xf[0][:, nlo:nlo + 512],
                                          x[0:128, nlo:nlo + 512])
                        nc.sync.dma_start(xf[1][:, nlo:nlo + 512],
                                          x[128:256, nlo:nlo + 512])
                    for r in range(4):
                        nc.vector.tensor_copy(
                            xp[r // 2][:, (r % 2) * N + lo:
                                       (r % 2) * N + lo + 512],
                            xf[r][:, lo:lo + 512])
                    qps = psQK.tile([128, 512], F32, tag="qk",
                                    name=f"qps{s}")
                    for pc in range(2):
                        mv = xp[pc][:].rearrange(
                            "p (i n) -> p i n", i=2)[:, :, lo:lo + 512]
                        st = wqkT8[pc][:].rearrange(
                            "p (i n) -> p i n", i=2)
                        nc.tensor.matmul(qps[:], st, mv,
                                         start=(pc == 0), stop=(pc == 1),
                                         perf_mode=DR)
                    # bias-add + bf16 cast on ACT (rides between exps)
                    nc.scalar.activation(qkb[:, lo:lo + 512], qps[:],
                                         AF.Identity, bias=bqk_s[:])
                    nc.sync.dma_start(
                        k2lo[:, lo:lo + 512], qkb[CQ:128, lo:lo + 512])
                    if s < 4:
                        nc.sync.dma_start(
                            q2hi[CQ:128, lo:lo + 512],
                            qkb[0:CQ, lo:lo + 512])

                # wq/wk transposed straight from f32; ptq shares the psQK
                # "qk" tag so it must be allocated before qps(0)
                ptq = [psQK.tile([128, 256], F32, tag="qk",
                                 name=f"ptq{i}") for i in range(2)]
                for cc in range(4):
                    nc.tensor.transpose(
                        ptq[cc // 2][:, (cc % 2) * 128:(cc % 2) * 128 + 128],
                        wqk_f[:, cc * 128:(cc + 1) * 128], id_f32[:])
                for pc in range(2):
                    nc.vector.tensor_copy(wqkT8[pc][:], ptq[pc][:])

                # -- slab 0 --
                slab_front(0)
                score_pair(expP, 0, 0)
                score_pair(expP, 0, 1)

                # -- slab 1 + wv prep (PE transposes from f32; fp8 copies
                #    on DVE) + epilogue constants --
                slab_front(1)
                for cc in range(4):
                    pt = psV.tile([128, C], F32, tag="v", name=f"ptv{cc}")
                    for r in range(4):
                        nc.tensor.transpose(
                            pt[:, r * 128:(r + 1) * 128],
                            wvf[r][:, cc * 128:(cc + 1) * 128],
                            id_f32[:],
                        )
                    nc.vector.tensor_copy(
                        wvTp[cc // 2][:, (cc % 2) * 512:(cc % 2) * 512 + 512],
                        pt[:])
                nc.vector.tensor_scalar_mul(gones[:], ones_f32[:], g_s[:])
                pg = psD.tile([128, 4], F32, tag="d", name="pg")
                nc.tensor.matmul(pg[:, 0:1], ones_f32[:], g_s[:],
                                 start=True, stop=True)
                nc.vector.tensor_copy(gammab[:], pg[:, 0:1])
                pbvT = psD.tile([128, 4], F32, tag="d", name="pbvT")
                for ct in range(4):
                    nc.tensor.matmul(
                        pbvT[:, ct:ct + 1],
                        bv_s[0:1, ct * 128:(ct + 1) * 128],
                        ones_f32[0:1, 0:1], start=True, stop=True)
                nc.vector.tensor_scalar_mul(gbv[:], pbvT[:], gammab[:])
                score_pair(expP, 0, 2)
                score_pair(expP, 0, 3)

                # -- slabs 2..7: steady state; v-pairs and the g0 denom
                #    chain lag two slabs/pairs behind --
                dt = psD.tile([128, 512], F32, tag="d", name="d0")
                for s in range(2, NS):
                    slab_front(s)
                    for j in (2 * s - 4, 2 * s - 3):
                        v_pair(j)
                    score_pair(expP, 0, 2 * s)
                    score_pair(expP, 0, 2 * s + 1)
                    dn_link(dt, expP, 2 * s - 4)
                    dn_link(dt, expP, 2 * s - 3)
                for j in (12, 13, 14, 15):
                    v_pair(j)
                    dn_link(dt, expP, j)

            # ============== P2: group slots + tail ========================
            with (
                tc.tile_pool(name="psAV", bufs=3, space="PSUM") as psAV,
                tc.tile_pool(name="small", bufs=2) as small,
                tc.tile_pool(name="yout", bufs=2) as yout,
            ):
                for g in range(N_G):
                    nxt = alloc_expP(g + 1) if g + 1 < N_G else None
                    dt_nxt = (psD.tile([128, 512], F32, tag="d",
                                       name=f"d{g + 1}")
                              if nxt is not None else None)
                    gcols = slice(g * 512, (g + 1) * 512)
                    dr = gdbs = av = None
                    for p in range(8):          # jj pairs
                        if nxt is not None:
                            score_pair(nxt, g + 1, 2 * p)
                            score_pair(nxt, g + 1, 2 * p + 1)
                            if p >= 1:
                                dn_link(dt_nxt, nxt, 2 * p - 2)
                                dn_link(dt_nxt, nxt, 2 * p - 1)
                        if p == 0:
                            # reciprocal runs on DVE hidden under the first
                            # AV half-chain; the gdb broadcast lands at p=1
                            dr = small.tile([1, 512], F32, tag="dr")
                            with nc.allow_low_precision(
                                    reason="approx 1/d; rescaled by gamma"):
                                nc.vector.reciprocal_approx_fast(
                                    dr[:], dt[0:1, :])
                        ct, half = p // 2, p % 2
                        if half == 0:
                            av = psAV.tile([128, 512], F32, tag="av",
                                           name=f"av{g}_{ct}")
                        for j in range(half * 8, half * 8 + 8):
                            vst = vP[j][:].rearrange(
                                "p (i n) -> p i n", i=2)[
                                :, :, ct * 128:(ct + 1) * 128]
                            nc.tensor.matmul(
                                av[:], vst,
                                expP[j][:].rearrange("p (i n) -> p i n",
                                                     i=2),
                                start=(j == 0), stop=(j == NJ - 1),
                                perf_mode=DR,
                            )
                        if p == 1:
                            gdb = psAV.tile([128, 512], F32, tag="av",
                                            name=f"gdb{g}")
                            nc.tensor.matmul(gdb[:], gones[:], dr[:],
                                             start=True, stop=True)
                            gdbs = small.tile([128, 512], F32, tag="gdbs",
                                              bufs=2)
                            nc.vector.tensor_copy(gdbs[:], gdb[:])
                        if half == 1:
                            tmp = yout.tile([128, 512], F32, tag="tmp")
                            nc.vector.tensor_tensor(tmp[:], av[:],
                                                    gdbs[:], MUL)
                            yo = yout.tile([128, 512], F32, tag="yo")
                            # yo = (tmp + gamma*bv) + x   (x f32 in SBUF)
                            nc.vector.scalar_tensor_tensor(
                                yo[:], tmp[:], gbv[:, ct:ct + 1],
                                xf[ct][:, gcols], ADD, ADD)
                            eng = nc.sync if ct % 2 == 0 else nc.gpsimd
                            eng.dma_start(
                                y[ct * 128:(ct + 1) * 128, gcols], yo[:])
                    if nxt is not None:
                        dn_link(dt_nxt, nxt, 14)
                        dn_link(dt_nxt, nxt, 15)
                    dt = dt_nxt
                    expP = nxt


def build_nc():
    nc = bacc.Bacc("TRN2", target_bir_lowering=False, debug=False,
                   num_devices=NCORES)
    x = nc.dram_tensor("x", [C, N], F32, kind="ExternalInput")
    wq = nc.dram_tensor("wq", [CQ, C], F32, kind="ExternalInput")
    wk = nc.dram_tensor("wk", [CQ, C], F32, kind="ExternalInput")
    wv = nc.dram_tensor("wv", [C, C], F32, kind="ExternalInput")
    bqk = nc.dram_tensor("bqk", [128, 1], F32, kind="ExternalInput")
    bv = nc.dram_tensor("bv", [1, C], F32, kind="ExternalInput")
    gamma = nc.dram_tensor("gamma", [1, 1], F32, kind="ExternalInput")
    y = nc.dram_tensor("y", [C, NH], F32, kind="ExternalOutput")
    with tile.TileContext(nc) as tc:
        _emit(tc, x.ap(), wq.ap(), wk.ap(), wv.ap(), bqk.ap(), bv.ap(),
              gamma.ap(), y.ap())
    nc.compile()
    return nc


def make_in_maps(inputs):
    xf = np.ascontiguousarray(
        np.asarray(inputs["x"], dtype=np.float32).reshape(B, C, N))
    wq = np.ascontiguousarray(np.asarray(inputs["wq"], dtype=np.float32))
    wk = np.ascontiguousarray(np.asarray(inputs["wk"], dtype=np.float32))
    wv = np.ascontiguousarray(np.asarray(inputs["wv"], dtype=np.float32))
    bqk = np.concatenate([
        np.asarray(inputs["bq"], dtype=np.float32),
        np.asarray(inputs["bk"], dtype=np.float32),
    ]).reshape(128, 1)
    bv = np.asarray(inputs["bv"], dtype=np.float32).reshape(1, C)
    gamma = np.asarray(inputs["gamma"], dtype=np.float32).reshape(1, 1)
    in_maps = []
    for i in range(NCORES):
        b, h = divmod(i, 2)
        xr = np.roll(xf[b], -h * NH, axis=1) if h else xf[b]
        in_maps.append({
            "x": np.ascontiguousarray(xr), "wq": wq, "wk": wk, "wv": wv,
            "bqk": bqk, "bv": bv, "gamma": gamma,
        })
    return in_maps


_NC = None


def _get_nc():
    global _NC
    if _NC is None:
        _NC = build_nc()
    return _NC


def kernel(**inputs):
    nc = _get_nc()
    in_maps = make_in_maps(inputs)
    res = bass_utils.run_bass_kernel_spmd(nc, in_maps, core_ids=list(range(NCORES)))
    yf = np.empty((B, C, N), dtype=np.float32)
    for i in range(NCORES):
        b, h = divmod(i, 2)
        yf[b][:, h * NH:(h + 1) * NH] = res.results[i]["y"]
    return yf.reshape(B, C, W, H)
